# revision 1
# baseline (speedup 1.0000x reference)
"""BitLinear forward on 8 Trainium2 NeuronCores.

Reference computation (see harness reference.py):
    xn      = rmsnorm(x) * norm_weight                     # per token over D
    w_scale = 1 / max(mean(|W|), 1e-5)                     # global scalar
    w_q     = clip(round(W * w_scale), -1, 1)              # ternary
    x_scale = 127 / max(max|xn| per token, 1e-5)
    x_q     = clip(round(xn * x_scale), -128, 127)
    y       = (x_q @ w_q.T) / (w_scale * x_scale)

Distribution (data-parallel tokens + sharded weight quantization):
  - tokens (B*S = 8192) sharded 1024/core; each core computes its tokens'
    full 4096-wide output rows.
  - W (4096x4096) sharded 512 rows/core for quantization; partial abs-sums
    are AllReduced (scalar) to form the global w_scale; each core quantizes
    + transposes its slice to bf16 and the four 128-row groups are
    AllGathered separately so matmuls can start after the first one lands.
  - The big matmul runs in bf16: x_q in [-127,127] and w_q in {-1,0,1} are
    integers, exactly representable in bf16, and PSUM accumulates fp32 with
    partial sums < 2^24, so the matmul is bit-exact integer arithmetic at
    full PE speed.
  - The matmul is split into two token-halves with the x-quantization of
    the second half emitted between them, so the PE never head-of-line
    blocks on transposes whose inputs (DVE/ACT quantize) aren't ready.

The round-to-nearest-even of jnp.round is reproduced with the fp32
+1.5*2^23 trick.
"""

import numpy as np

# ---------------------------------------------------------------- constants
R = 8  # cores
B, S, D = 4, 2048, 4096
N = 4096  # out features
TOK = (B * S) // R  # tokens per core (1024)
NS = N // R  # weight rows per core (512)
HALF = D // 2  # free-dim half tile (2048)
C_ROUND = 12582912.0  # 1.5 * 2^23: fp32 add rounds to int, ties-to-even
CLIP_W = 1.4999999  # largest fp32 < 1.5: pre-round clip for ternary weights
EPS_NORM = 1e-5
Q_EPS = 1e-5

NWT = NS // 128  # weight row tiles per core (4)
NTT = TOK // 128  # token tiles per core (8)
NDC = D // 128  # contraction chunks (32)

_CACHED = {}


def _legalize_waits(bir_bytes):
    """Split multi-wait BIR instructions into single-wait EventSemaphore
    chains: the walrus build here accepts at most one sync-wait command per
    instruction, while Tile's sem-assignment emits multi-wait joins."""
    import json

    bir = json.loads(bir_bytes)
    for fn in bir.get("functions", []):
        for bb in fn.get("blocks", []):
            new_insts = []
            for inst in bb.get("instructions", []):
                si = inst.get("sync_info")
                waits = (si or {}).get("on_wait") or []
                if len(waits) > 1:
                    movable = [w for w in waits if w.get("sync_type") == "semaphore"]
                    fixed = [w for w in waits if w.get("sync_type") != "semaphore"]
                    keep, hoist = (
                        (fixed, movable) if fixed else ([movable[-1]], movable[:-1])
                    )
                    if len(keep) > 1:
                        raise RuntimeError(
                            f"{inst.get('name')}: {len(keep)} non-hoistable waits"
                        )
                    for k, w in enumerate(hoist):
                        new_insts.append(
                            {
                                "debug": inst.get("debug", 0),
                                "engine": inst["engine"],
                                "ins": [],
                                "name": f"{inst['name']}_hw{k}",
                                "opcode": "EventSemaphore",
                                "outs": [],
                                "sync_info": {"on_update": [], "on_wait": [w]},
                            }
                        )
                    si["on_wait"] = keep
                new_insts.append(inst)
            bb["instructions"] = new_insts
    return json.dumps(bir).encode()


def _build(with_g, use_coll=True, do_mm=True, do_prep=True, nrep=1, wait_hint=True):
    import concourse.bass as bass
    import concourse.mybir as mybir
    import concourse.tile as tile
    from concourse.bass import ts
    from concourse.masks import make_identity

    f32 = mybir.dt.float32
    bf16 = mybir.dt.bfloat16
    MULT = mybir.AluOpType.mult
    ADD = mybir.AluOpType.add
    MAX = mybir.AluOpType.max
    SUB = mybir.AluOpType.subtract
    MIN = mybir.AluOpType.min
    X_AX = mybir.AxisListType.X
    AF = mybir.ActivationFunctionType
    GROUP = [list(range(R))]

    nc = bass.Bass()
    xp = nc.declare_dram_parameter("x", [TOK, D], f32, isOutput=False)
    wp = nc.declare_dram_parameter("w", [NS, D], f32, isOutput=False)
    gp = nc.declare_dram_parameter("g", [D], f32, isOutput=False)
    yp = nc.declare_dram_parameter("y", [TOK, N], f32, isOutput=True)

    with tile.TileContext(nc) as tc:
        with (
            tc.tile_pool(name="persist", bufs=1) as pp,
            tc.tile_pool(name="io_w", bufs=2) as io_w,
            tc.tile_pool(name="io_x", bufs=3) as io_x,
            tc.tile_pool(name="scr_w", bufs=2) as scr_w,
            tc.tile_pool(name="scr_x", bufs=2) as scr_x,
            tc.tile_pool(name="qb_w", bufs=1) as qb_w,
            tc.tile_pool(name="qb_x", bufs=2) as qb_x,
            tc.tile_pool(name="small", bufs=24) as sp,
            tc.tile_pool(name="stage", bufs=2) as stp,
            tc.tile_pool(name="wcol", bufs=6) as wcp,
            tc.tile_pool(name="yout", bufs=2) as ypool,
            tc.tile_pool(name="ps_t", bufs=2, space="PSUM") as ps_t,
            tc.tile_pool(name="ps_mm", bufs=3, space="PSUM") as ps_mm,
            tc.tile_pool(name="ps_s", bufs=1, space="PSUM") as ps_s,
            tc.tile_pool(name="dram", bufs=1, space="DRAM") as dram,
        ):
            # ---- persistent tiles
            xqT = pp.tile([128, NDC, TOK], bf16, name="xqT")
            ident = pp.tile([128, 128], bf16, name="ident")
            make_identity(nc, ident[:])
            ones_col = pp.tile([128, 1], f32, name="ones_col")
            nc.vector.memset(ones_col[:], 1.0)
            c2_all = pp.tile([128, NTT], f32, name="c2_all")
            s_rep = pp.tile([128, 1], f32, name="s_rep")
            m_rep = pp.tile([128, 1], f32, name="m_rep")
            wsc_rep = pp.tile([128, 1], f32, name="wsc_rep")
            dq_rep = pp.tile([128, 1], f32, name="dq_rep")
            eps_rep = pp.tile([128, 1], f32, name="eps_rep")
            nc.vector.memset(eps_rep[:], EPS_NORM)
            if with_g:
                g_rep = pp.tile([128, D], f32, name="g_rep")
                nc.sync.dma_start(g_rep[:], gp[:].to_broadcast([128, D]))

            for _rep in range(nrep):
                # ---- DRAM scratch (per rep: Shared tensors are single-writer)
                ws_in = dram.tile([1, 1], f32, name="ws_in")
                ws_out = dram.tile([1, 1], f32, addr_space="Shared", name="ws_out")
                wqTl = [
                    dram.tile([D, 128], bf16, name=f"wqTl{i}_{_rep}", uniquify=False)
                    for i in range(NWT)
                ]
                wqTa = [
                    dram.tile(
                        [R, D, 128],
                        bf16,
                        addr_space="Shared",
                        name=f"wqTa{i}_{_rep}",
                        uniquify=False,
                    )
                    for i in range(NWT)
                ]

                # ============ W1: partial abs-sum of the W slice ============
                parts = []
                if do_prep:
                    for i in range(NWT):
                        for h in range(2):
                            w_t = io_w.tile([128, HALF], f32, tag="iow", name="w_t")
                            nc.sync.dma_start(w_t[:], wp[ts(i, 128), ts(h, HALF)])
                            part = sp.tile([128, 1], f32, tag="sm", name="part")
                            junk = scr_w.tile(
                                [128, HALF], f32, tag="scw", name="junk"
                            )
                            nc.scalar.activation(
                                junk[:], w_t[:], AF.Abs, accum_out=part[:]
                            )
                            parts.append(part)
                else:
                    p0 = sp.tile([128, 1], f32, tag="sm", name="p0")
                    nc.vector.memset(p0[:], 1.0)
                    parts = [p0]
                while len(parts) > 1:  # pairwise tree sum
                    nxt = []
                    for a, b_ in zip(parts[::2], parts[1::2]):
                        s2 = sp.tile([128, 1], f32, tag="sm", name="s2")
                        nc.vector.tensor_add(s2[:], a[:], b_[:])
                        nxt.append(s2)
                    parts = nxt
                acc = parts[0]
                pst_s = ps_s.tile([1, 1], f32, name="pst_s")
                nc.tensor.matmul(
                    pst_s[:], lhsT=acc[:], rhs=ones_col[:], start=True, stop=True
                )
                sb_tot = sp.tile([1, 1], f32, tag="one", name="sb_tot")
                nc.scalar.copy(sb_tot[:], pst_s[:])
                nc.sync.dma_start(ws_in[:], sb_tot[:])
                if use_coll:
                    nc.gpsimd.collective_compute(
                        "AllReduce",
                        ADD,
                        replica_groups=GROUP,
                        ins=[ws_in[:]],
                        outs=[ws_out[:]],
                    )
                    nc.sync.dma_start(s_rep[:], ws_out[:].to_broadcast([128, 1]))
                else:
                    nc.sync.dma_start(s_rep[:], ws_in[:].to_broadcast([128, 1]))

                # w_scale machinery (per-partition replicated):
                #   m_rep  = max(mean|W|, Q_EPS)     (= 1/w_scale)
                #   wsc_rep= 1/m_rep                 (= w_scale)
                #   dq_rep = m_rep/127               (= 1/(127*w_scale))
                nc.vector.tensor_scalar(
                    m_rep[:], s_rep[:], 1.0 / (N * D), Q_EPS, op0=MULT, op1=MAX
                )
                nc.vector.reciprocal(wsc_rep[:], m_rep[:])
                nc.vector.tensor_scalar_mul(dq_rep[:], m_rep[:], 1.0 / 127.0)

                # ============ W2: quantize + transpose slice, AllGather =====
                for i in range(NWT) if do_prep else []:
                    for h in range(2):
                        w_t = io_w.tile([128, HALF], f32, tag="iow", name="w_t2")
                        nc.sync.dma_start(w_t[:], wp[ts(i, 128), ts(h, HALF)])
                        u = scr_w.tile([128, HALF], f32, tag="scw", name="u")
                        nc.vector.tensor_scalar(
                            u[:], w_t[:], wsc_rep[:], CLIP_W, op0=MULT, op1=MIN
                        )
                        nc.vector.tensor_scalar(
                            u[:], u[:], -CLIP_W, C_ROUND, op0=MAX, op1=ADD
                        )
                        wq = qb_w.tile([128, HALF], bf16, tag="qbw", name="wq")
                        nc.vector.tensor_scalar(wq[:], u[:], C_ROUND, None, op0=SUB)
                        for bk in range(2):  # 2 psum banks of 8 transposes
                            pst = ps_t.tile(
                                [128, 1024], bf16, tag="pst", name="pstw"
                            )
                            for j4 in range(8):
                                j = bk * 8 + j4
                                nc.tensor.transpose(
                                    pst[:, ts(j4, 128)], wq[:, ts(j, 128)], ident[:]
                                )
                            stg = stp.tile([128, 1024], bf16, tag="stg", name="stg")
                            nc.scalar.copy(stg[:], pst[:])
                            dc0 = h * (HALF // 128) + bk * 8
                            dst = wqTl[i][:].rearrange("(dc p) q -> p dc q", p=128)[
                                :, dc0 : dc0 + 8, :
                            ]
                            nc.sync.dma_start(
                                dst, stg[:].rearrange("p (j q) -> p j q", j=8)
                            )
                    if use_coll:
                        nc.gpsimd.collective_compute(
                            "AllGather",
                            mybir.AluOpType.bypass,
                            replica_groups=GROUP,
                            ins=[wqTl[i][:]],
                            outs=[wqTa[i][:]],
                        )

                # ============ X(t): rmsnorm + int8 quantize + transpose =====
                def emit_x(t, wait_ms=None):
                    if wait_ms is not None:
                        with tc.tile_wait_until(wait_ms):
                            emit_x_body(t)
                    else:
                        emit_x_body(t)

                def emit_x_body(t):
                    srcs, mss, amaxs = [], [], []
                    for h in range(2):
                        x_t = io_x.tile([128, HALF], f32, tag="iox", name="x_t")
                        nc.sync.dma_start(x_t[:], xp[ts(t, 128), ts(h, HALF)])
                        sq = scr_x.tile([128, HALF], f32, tag="scx", name="sq")
                        ms_h = sp.tile([128, 1], f32, tag="sm", name="ms_h")
                        # sq <- x*x (scratch), ms_h <- sum(x*x)
                        nc.scalar.activation(
                            sq[:], x_t[:], AF.Square, accum_out=ms_h[:]
                        )
                        if with_g:
                            nc.vector.tensor_mul(
                                sq[:], x_t[:], g_rep[:, ts(h, HALF)]
                            )  # sq <- x*g
                            src = sq
                        else:
                            src = x_t
                        srcs.append((src, sq))
                        am_h = sp.tile([128, 1], f32, tag="sm", name="am_h")
                        nc.vector.tensor_reduce(
                            am_h[:],
                            src[:],
                            axis=X_AX,
                            op=MAX,
                            apply_absolute_value=True,
                        )
                        mss.append(ms_h)
                        amaxs.append(am_h)
                    ms = sp.tile([128, 1], f32, tag="sm", name="ms")
                    nc.vector.tensor_add(ms[:], mss[0][:], mss[1][:])
                    amax = sp.tile([128, 1], f32, tag="sm", name="amax")
                    nc.vector.tensor_tensor(amax[:], amaxs[0][:], amaxs[1][:], op=MAX)
                    # r = 1/sqrt(ms/D + eps)
                    sdev = sp.tile([128, 1], f32, tag="sm", name="sdev")
                    nc.scalar.activation(
                        sdev[:], ms[:], AF.Sqrt, bias=eps_rep[:], scale=1.0 / D
                    )
                    r = sp.tile([128, 1], f32, tag="sm", name="r")
                    nc.vector.reciprocal(r[:], sdev[:])
                    # x_scale = 127/max(amax*r, eps); c = r*x_scale
                    amn = sp.tile([128, 1], f32, tag="sm", name="amn")
                    nc.vector.tensor_mul(amn[:], amax[:], r[:])
                    amc = sp.tile([128, 1], f32, tag="sm", name="amc")
                    nc.vector.tensor_scalar_max(amc[:], amn[:], Q_EPS)
                    inv = sp.tile([128, 1], f32, tag="sm", name="inv")
                    nc.vector.reciprocal(inv[:], amc[:])
                    rc = sp.tile([128, 1], f32, tag="sm", name="rc")
                    nc.vector.tensor_mul(rc[:], r[:], inv[:])
                    c_t = sp.tile([128, 1], f32, tag="sm", name="c_t")
                    nc.vector.tensor_scalar_mul(c_t[:], rc[:], 127.0)
                    # c2 = 1/(w_scale * x_scale) = amc * m'/127
                    nc.vector.tensor_mul(c2_all[:, t : t + 1], amc[:], dq_rep[:])
                    for h in range(2):
                        src, sq = srcs[h]
                        # v = src*c + C_ROUND (fp32: rounds to int, ties-even)
                        nc.vector.tensor_scalar(
                            sq[:], src[:], c_t[:], C_ROUND, op0=MULT, op1=ADD
                        )
                        xq = qb_x.tile([128, HALF], bf16, tag="qbx", name="xq")
                        nc.vector.tensor_scalar(xq[:], sq[:], C_ROUND, None, op0=SUB)
                        for bk in range(2):
                            pst = ps_t.tile(
                                [128, 1024], bf16, tag="pst", name="pstx"
                            )
                            for j4 in range(8):
                                j = bk * 8 + j4
                                nc.tensor.transpose(
                                    pst[:, ts(j4, 128)], xq[:, ts(j, 128)], ident[:]
                                )
                            dc0 = h * (HALF // 128) + bk * 8
                            nc.scalar.copy(
                                xqT[:, dc0 : dc0 + 8, ts(t, 128)],
                                pst[:].rearrange("p (j q) -> p j q", j=8),
                            )

                # ============ MM chunk (i, rh) over a token range ===========
                # y columns {r*512 + i*128 + q} for ranks r in rh*4..rh*4+4:
                # each chunk depends on AG_i only.
                def emit_mm(i, rh, t_range):
                    wcs = []
                    for s4 in range(4):
                        wc = wcp.tile([128, 8, 512], bf16, tag="wc", name="wc")
                        for rr in range(4):
                            r = rh * 4 + rr
                            wsrc = wqTa[i][r] if use_coll else wqTl[i][:]
                            wsrc = wsrc.rearrange("(dc p) q -> p dc q", p=128)[
                                :, s4 * 8 : (s4 + 1) * 8, :
                            ]
                            nc.sync.dma_start(wc[:, :, ts(rr, 128)], wsrc)
                        wcs.append(wc)
                    for t in t_range:
                        pmm = ps_mm.tile([128, 512], f32, tag="pmm", name="pmm")
                        for dc in range(NDC):
                            nc.tensor.matmul(
                                pmm[:],
                                lhsT=xqT[:, dc, ts(t, 128)],
                                rhs=wcs[dc // 8][:, dc % 8, :],
                                start=(dc == 0),
                                stop=(dc == NDC - 1),
                            )
                        y_sb = ypool.tile([128, 512], f32, tag="y", name="y_sb")
                        nc.scalar.activation(
                            y_sb[:], pmm[:], AF.Copy, scale=c2_all[:, t : t + 1]
                        )
                        ydst = yp[ts(t, 128), :].rearrange(
                            "p (r i q) -> p r i q", r=R, i=NWT
                        )[:, rh * 4 : (rh + 1) * 4, i, :]
                        nc.sync.dma_start(
                            ydst, y_sb[:].rearrange("p (r q) -> p r q", r=4)
                        )

                # Interleaved emission: X half A, MM half A, X half B
                # (pushed in scheduler sim-time past MM half A via
                # tile_wait_until so the PE stream interleaves as
                # [XT(0..3), mmA, XT(4..7), mmB] — the scheduler's
                # collective cost model would otherwise place every X
                # transpose before the first matmul), then MM half B.
                halves = [range(0, NTT // 2), range(NTT // 2, NTT)]
                base_ms = 2.0 * _rep
                if do_prep:
                    for t in halves[0]:
                        emit_x(t)
                if do_mm:
                    for i in range(NWT):
                        for rh in range(2):
                            emit_mm(i, rh, halves[0])
                if do_prep:
                    for t in halves[1]:
                        emit_x(
                            t,
                            wait_ms=(base_ms + 0.95 + 0.05 * (t - NTT // 2))
                            if wait_hint
                            else None,
                        )
                if do_mm:
                    for i in range(NWT):
                        for rh in range(2):
                            emit_mm(i, rh, halves[1])

    orig = nc.to_json_bytes

    def patched():
        return _legalize_waits(orig())

    nc.to_json_bytes = patched
    return nc


def _get_nc(with_g, **kw):
    key = ("nc", with_g, tuple(sorted(kw.items())))
    if key not in _CACHED:
        _CACHED[key] = _build(with_g, **kw)
    return _CACHED[key]


def kernel(x, weight, norm_weight):
    from concourse.bass_utils import run_bass_kernel_spmd

    x = np.ascontiguousarray(x, dtype=np.float32)
    weight = np.ascontiguousarray(weight, dtype=np.float32)
    norm_weight = np.ascontiguousarray(norm_weight, dtype=np.float32)
    xf = x.reshape(B * S, D)

    with_g = not bool(np.all(norm_weight == 1.0))
    nc = _get_nc(with_g)

    in_maps = []
    for i in range(R):
        in_maps.append(
            {
                "x": xf[i * TOK : (i + 1) * TOK],
                "w": weight[i * NS : (i + 1) * NS],
                "g": norm_weight,
            }
        )
    res = run_bass_kernel_spmd(nc, in_maps, list(range(R)))
    y = np.concatenate([res.results[i]["y"] for i in range(R)], axis=0)
    return y.reshape(B, S, N)


if __name__ == "__main__":
    rng = np.random.default_rng(0)
    x = rng.standard_normal((B, S, D), dtype=np.float32)
    w = (rng.standard_normal((N, D), dtype=np.float32) * np.sqrt(2.0 / D)).astype(
        np.float32
    )
    g = np.ones(D, dtype=np.float32)
    y = kernel(x, w, g)
    print("ran", y.shape, y.dtype)



# revision 7
# speedup vs baseline: 1.2083x; 1.2083x over previous
"""BitLinear forward on 8 Trainium2 NeuronCores.

Reference computation (see harness reference.py):
    xn      = rmsnorm(x) * norm_weight                     # per token over D
    w_scale = 1 / max(mean(|W|), 1e-5)                     # global scalar
    w_q     = clip(round(W * w_scale), -1, 1)              # ternary
    x_scale = 127 / max(max|xn| per token, 1e-5)
    x_q     = clip(round(xn * x_scale), -128, 127)
    y       = (x_q @ w_q.T) / (w_scale * x_scale)

Distribution: data-parallel over tokens (1024/core), weight REPLICATED.
The host passes W pre-transposed (wt = W.T, [D, N] row-major) to every
core, so each core:
  - computes the global |W| abs-sum from its own disjoint 512-row slice
    of wt, AllReduces the scalar (the only collective on the critical
    path; a tiny warm-up AllReduce is issued at kernel start so the
    real one doesn't pay collective cold-start / launch skew),
  - rmsnorms + int8-quantizes + PE-transposes its 1024 tokens into a
    resident xqT [128, 32dc, 1024tok] bf16 SBUF tile, all before the
    matmul phase starts - no mid-matmul x stalls,
  - streams wt from its own DRAM in [128, 4dc, 512out] fp32 stages,
    quantizing to ternary bf16 on the fly (DVE round via the fp32
    +1.5*2^23 trick, then ACT Sign: clip(round(v),-1,1) == sign of the
    rounded integer; all ACT funcs used live in one table set),
  - runs 8 out-chunks x 8 token-tiles x 32 accumulating bf16 matmuls
    (exact integer arithmetic: x_q in [-127,127], w_q in {-1,0,1},
    fp32 PSUM partial sums < 2^24).
No AllGather, no quantized-W DRAM round-trip.
"""

import numpy as np

# ---------------------------------------------------------------- constants
R = 8  # cores
B, S, D = 4, 2048, 4096
N = 4096  # out features
TOK = (B * S) // R  # tokens per core (1024)
NS = N // R  # wslice rows per core (512)
HALF = D // 2  # x free-dim half tile (2048)
NTT = TOK // 128  # token tiles per core (8)
NDC = D // 128  # contraction chunks (32)
NCH = N // 512  # output chunks (8)
NST = 8  # W stages per chunk (4 dc each)
SDC = NDC // NST  # dc per stage (4)
C_ROUND = 12582912.0  # 1.5 * 2^23: fp32 add rounds to int, ties-to-even
EPS_NORM = 1e-5
Q_EPS = 1e-5

_CACHED = {}


def _legalize_waits(bir_bytes):
    """Split multi-wait BIR instructions into single-wait EventSemaphore
    chains: the walrus build here accepts at most one sync-wait command per
    instruction, while Tile's sem-assignment emits multi-wait joins."""
    import json

    bir = json.loads(bir_bytes)
    for fn in bir.get("functions", []):
        for bb in fn.get("blocks", []):
            new_insts = []
            for inst in bb.get("instructions", []):
                si = inst.get("sync_info")
                waits = (si or {}).get("on_wait") or []
                if len(waits) > 1:
                    movable = [w for w in waits if w.get("sync_type") == "semaphore"]
                    fixed = [w for w in waits if w.get("sync_type") != "semaphore"]
                    keep, hoist = (
                        (fixed, movable) if fixed else ([movable[-1]], movable[:-1])
                    )
                    if len(keep) > 1:
                        raise RuntimeError(
                            f"{inst.get('name')}: {len(keep)} non-hoistable waits"
                        )
                    for k, w in enumerate(hoist):
                        new_insts.append(
                            {
                                "debug": inst.get("debug", 0),
                                "engine": inst["engine"],
                                "ins": [],
                                "name": f"{inst['name']}_hw{k}",
                                "opcode": "EventSemaphore",
                                "outs": [],
                                "sync_info": {"on_update": [], "on_wait": [w]},
                            }
                        )
                    si["on_wait"] = keep
                new_insts.append(inst)
            bb["instructions"] = new_insts
    return json.dumps(bir).encode()


def _build(with_g):
    import concourse.bass as bass
    import concourse.mybir as mybir
    import concourse.tile as tile
    from concourse.bass import ts
    from concourse.masks import make_identity

    f32 = mybir.dt.float32
    bf16 = mybir.dt.bfloat16
    MULT = mybir.AluOpType.mult
    ADD = mybir.AluOpType.add
    MAX = mybir.AluOpType.max
    X_AX = mybir.AxisListType.X
    AF = mybir.ActivationFunctionType
    GROUP = [list(range(R))]

    nc = bass.Bass()
    xp = nc.declare_dram_parameter("x", [TOK, D], f32, isOutput=False)
    wtp = nc.declare_dram_parameter("wt", [D, N], f32, isOutput=False)
    wsl = nc.declare_dram_parameter("wsl", [NS, D], f32, isOutput=False)
    gp = nc.declare_dram_parameter("g", [D], f32, isOutput=False)
    yp = nc.declare_dram_parameter("y", [TOK, N], f32, isOutput=True)

    wcb_bufs = 12 if with_g else 16
    iox_bufs = 2 if with_g else 3

    with tile.TileContext(nc) as tc:
        with (
            tc.tile_pool(name="persist", bufs=1) as pp,
            tc.tile_pool(name="io_x", bufs=iox_bufs) as io_x,
            tc.tile_pool(name="scr_x", bufs=2) as scr_x,
            tc.tile_pool(name="wst", bufs=3) as wst,
            tc.tile_pool(name="wcb", bufs=wcb_bufs) as wcb_pool,
            tc.tile_pool(name="small", bufs=24) as sp,
            tc.tile_pool(name="yout", bufs=2) as ypool,
            tc.tile_pool(name="ps_t", bufs=2, space="PSUM") as ps_t,
            tc.tile_pool(name="ps_mm", bufs=4, space="PSUM") as ps_mm,
            tc.tile_pool(name="ps_s", bufs=1, space="PSUM") as ps_s,
            tc.tile_pool(name="dram", bufs=1, space="DRAM") as dram,
        ):
            # ---- persistent tiles
            xqT = pp.tile([128, NDC, TOK], bf16, name="xqT")
            ident = pp.tile([128, 128], f32, name="ident")
            make_identity(nc, ident[:])
            ones_col = pp.tile([128, 1], f32, name="ones_col")
            nc.vector.memset(ones_col[:], 1.0)
            amc_all = pp.tile([128, NTT], f32, name="amc_all")
            c2_all = pp.tile([128, NTT], f32, name="c2_all")
            s_rep = pp.tile([128, 1], f32, name="s_rep")
            m_rep = pp.tile([128, 1], f32, name="m_rep")
            wsc_rep = pp.tile([128, 1], f32, name="wsc_rep")
            dq_rep = pp.tile([128, 1], f32, name="dq_rep")
            eps_rep = pp.tile([128, 1], f32, name="eps_rep")
            nc.vector.memset(eps_rep[:], EPS_NORM)
            nround_rep = pp.tile([128, 1], f32, name="nround_rep")
            nc.vector.memset(nround_rep[:], -C_ROUND)
            if with_g:
                g_rep = pp.tile([128, D], f32, name="g_rep")
                nc.sync.dma_start(g_rep[:], gp[:].to_broadcast([128, D]))

            # ---- DRAM scratch
            warm_in = dram.tile([1, 1], f32, name="warm_in")
            warm_out = dram.tile([1, 1], f32, addr_space="Shared", name="warm_out")
            ws_in = dram.tile([1, 1], f32, name="ws_in")
            ws_out = dram.tile([1, 1], f32, addr_space="Shared", name="ws_out")

            # ---- warm-up collective: absorbs launch skew + collective
            # cold-start so the real w_scale AllReduce is mesh-only.
            wz = sp.tile([1, 1], f32, tag="one", name="wz")
            nc.vector.memset(wz[:], 0.0)
            nc.sync.dma_start(warm_in[:], wz[:])
            nc.gpsimd.collective_compute(
                "AllReduce",
                ADD,
                replica_groups=GROUP,
                ins=[warm_in[:]],
                outs=[warm_out[:]],
            )

            # ---- W1: partial |W| abs-sum over this core's disjoint slice
            parts = []
            for i in range(NS // 128):
                for h in range(2):
                    w_t = wst.tile([128, SDC, 512], f32, tag="wst", name="ws_t")
                    fl = w_t[:].rearrange("p j q -> p (j q)")
                    nc.sync.dma_start(fl, wsl[ts(i, 128), ts(h, HALF)])
                    part = sp.tile([128, 1], f32, tag="sm", name="part")
                    nc.scalar.activation(fl, fl, AF.Abs, accum_out=part[:])
                    parts.append(part)

            # ---- X(t): rmsnorm + int8 quantize + transpose into xqT
            def emit_x(t):
                srcs = []
                mss, amaxs = [], []
                for h in range(2):
                    x_t = io_x.tile([128, HALF], f32, tag="iox", name="x_t")
                    nc.sync.dma_start(x_t[:], xp[ts(t, 128), ts(h, HALF)])
                    ms_h = sp.tile([128, 1], f32, tag="sm", name="ms_h")
                    sq = scr_x.tile([128, HALF], f32, tag="scx", name="sq")
                    # sq <- x*x (scratch, overwritten later), ms_h <- sum(x^2)
                    nc.scalar.activation(sq[:], x_t[:], AF.Square, accum_out=ms_h[:])
                    if with_g:
                        nc.vector.tensor_mul(sq[:], x_t[:], g_rep[:, ts(h, HALF)])
                        src = sq
                    else:
                        src = x_t
                    srcs.append((src, sq))
                    am_h = sp.tile([128, 1], f32, tag="sm", name="am_h")
                    nc.vector.tensor_reduce(
                        am_h[:], src[:], axis=X_AX, op=MAX, apply_absolute_value=True
                    )
                    mss.append(ms_h)
                    amaxs.append(am_h)
                ms = sp.tile([128, 1], f32, tag="sm", name="ms")
                nc.vector.tensor_add(ms[:], mss[0][:], mss[1][:])
                amax = sp.tile([128, 1], f32, tag="sm", name="amax")
                nc.vector.tensor_tensor(amax[:], amaxs[0][:], amaxs[1][:], op=MAX)
                # r = 1/sqrt(ms/D + eps)
                sdev = sp.tile([128, 1], f32, tag="sm", name="sdev")
                nc.scalar.activation(
                    sdev[:], ms[:], AF.Sqrt, bias=eps_rep[:], scale=1.0 / D
                )
                r = sp.tile([128, 1], f32, tag="sm", name="r")
                nc.vector.reciprocal(r[:], sdev[:])
                # amc = max(amax*r, eps) = max(max|xn|, eps);  c = r*127/amc
                amn = sp.tile([128, 1], f32, tag="sm", name="amn")
                nc.vector.tensor_mul(amn[:], amax[:], r[:])
                nc.vector.tensor_scalar_max(amc_all[:, t : t + 1], amn[:], Q_EPS)
                inv = sp.tile([128, 1], f32, tag="sm", name="inv")
                nc.vector.reciprocal(inv[:], amc_all[:, t : t + 1])
                rc = sp.tile([128, 1], f32, tag="sm", name="rc")
                nc.vector.tensor_mul(rc[:], r[:], inv[:])
                c_t = sp.tile([128, 1], f32, tag="sm", name="c_t")
                nc.vector.tensor_scalar_mul(c_t[:], rc[:], 127.0)
                for h in range(2):
                    src, sq = srcs[h]
                    # v = src*c + C_ROUND  (fp32: rounds to int, ties-even)
                    nc.vector.tensor_scalar(
                        sq[:], src[:], c_t[:], C_ROUND, op0=MULT, op1=ADD
                    )
                    # transpose v (fp32) via PE; subtract C_ROUND during the
                    # PSUM->SBUF copy (ACT, fp32->bf16): xqT gets exact int8
                    for bk in range(4):
                        pst = ps_t.tile([128, 512], f32, tag="pst", name="pstx")
                        for j4 in range(4):
                            j = bk * 4 + j4
                            nc.tensor.transpose(
                                pst[:, ts(j4, 128)], sq[:, ts(j, 128)], ident[:]
                            )
                        dc0 = h * (HALF // 128) + bk * 4
                        nc.scalar.activation(
                            xqT[:, dc0 : dc0 + 4, ts(t, 128)],
                            pst[:].rearrange("p (j q) -> p j q", j=4),
                            AF.Copy,
                            bias=-C_ROUND,
                        )

            # ---- Wq(c): stream chunk c of wt, quantize to ternary bf16
            def emit_wq(c):
                tiles = []
                for g in range(NST):
                    ws = wst.tile([128, SDC, 512], f32, tag="wst", name="ws_t")
                    src = wtp[ts(g, SDC * 128), ts(c, 512)].rearrange(
                        "(j p) q -> p j q", p=128
                    )
                    nc.sync.dma_start(ws[:], src)
                    flat = ws[:].rearrange("p j q -> p (j q)")
                    # v = w*wsc + C_ROUND (fp32 round); wq = Sign(v - C_ROUND)
                    nc.vector.tensor_scalar(
                        flat, flat, wsc_rep[:], C_ROUND, op0=MULT, op1=ADD
                    )
                    wq = wcb_pool.tile([128, SDC, 512], bf16, tag="wcb", name="wq")
                    nc.scalar.activation(
                        wq[:].rearrange("p j q -> p (j q)"),
                        flat,
                        AF.Sign,
                        bias=nround_rep[:],
                    )
                    tiles.append(wq)
                return tiles

            # ---- MM(c): 8 token tiles x 32 accumulating matmuls
            def emit_mm(c, wq_tiles):
                for t in range(NTT):
                    pmm = ps_mm.tile([128, 512], f32, tag="pmm", name="pmm")
                    for g in range(NST):
                        for j in range(SDC):
                            dc = g * SDC + j
                            nc.tensor.matmul(
                                pmm[:],
                                lhsT=xqT[:, dc, ts(t, 128)],
                                rhs=wq_tiles[g][:, j, :],
                                start=(dc == 0),
                                stop=(dc == NDC - 1),
                            )
                    y_sb = ypool.tile([128, 512], f32, tag="y", name="y_sb")
                    nc.scalar.activation(
                        y_sb[:], pmm[:], AF.Copy, scale=c2_all[:, t : t + 1]
                    )
                    nc.sync.dma_start(yp[ts(t, 128), ts(c, 512)], y_sb[:])

            # ================= emission schedule =================
            for t in range(3):
                emit_x(t)

            # finish the abs-sum -> scalar AllReduce (PE reduction emitted
            # here so the x0-2 transposes aren't stuck behind its sem wait)
            while len(parts) > 1:
                nxt = []
                for a, b_ in zip(parts[::2], parts[1::2]):
                    s2 = sp.tile([128, 1], f32, tag="sm", name="s2")
                    nc.vector.tensor_add(s2[:], a[:], b_[:])
                    nxt.append(s2)
                if len(parts) % 2:
                    nxt.append(parts[-1])
                parts = nxt
            pst_s = ps_s.tile([1, 1], f32, name="pst_s")
            nc.tensor.matmul(
                pst_s[:], lhsT=parts[0][:], rhs=ones_col[:], start=True, stop=True
            )
            sb_tot = sp.tile([1, 1], f32, tag="one", name="sb_tot")
            nc.scalar.copy(sb_tot[:], pst_s[:])
            nc.sync.dma_start(ws_in[:], sb_tot[:])
            nc.gpsimd.collective_compute(
                "AllReduce",
                ADD,
                replica_groups=GROUP,
                ins=[ws_in[:]],
                outs=[ws_out[:]],
            )
            nc.sync.dma_start(s_rep[:], ws_out[:].to_broadcast([128, 1]))

            for t in range(3, NTT):
                emit_x(t)

            # w_scale machinery (replicated per partition):
            #   m_rep  = max(mean|W|, Q_EPS)   (= 1/w_scale)
            #   wsc_rep= 1/m_rep               (= w_scale)
            #   dq_rep = m_rep/127             (= 1/(127*w_scale))
            nc.vector.tensor_scalar(
                m_rep[:], s_rep[:], 1.0 / (N * D), Q_EPS, op0=MULT, op1=MAX
            )
            nc.vector.reciprocal(wsc_rep[:], m_rep[:])
            nc.vector.tensor_scalar_mul(dq_rep[:], m_rep[:], 1.0 / 127.0)
            # c2 = 1/(w_scale*x_scale) per token tile
            for t in range(NTT):
                nc.vector.tensor_mul(
                    c2_all[:, t : t + 1], amc_all[:, t : t + 1], dq_rep[:]
                )

            wq_c = emit_wq(0)
            wq_n = emit_wq(1)
            for c in range(NCH):
                cur = wq_c
                wq_c = wq_n
                if c + 2 < NCH:
                    wq_n = emit_wq(c + 2)
                emit_mm(c, cur)

    orig = nc.to_json_bytes

    def patched():
        return _legalize_waits(orig())

    nc.to_json_bytes = patched
    return nc


def _get_nc(with_g):
    key = ("nc", with_g)
    if key not in _CACHED:
        _CACHED[key] = _build(with_g)
    return _CACHED[key]


def make_in_maps(x, weight, norm_weight):
    x = np.ascontiguousarray(x, dtype=np.float32)
    weight = np.ascontiguousarray(weight, dtype=np.float32)
    norm_weight = np.ascontiguousarray(norm_weight, dtype=np.float32)
    xf = x.reshape(B * S, D)
    wt = np.ascontiguousarray(weight.T)
    in_maps = []
    for i in range(R):
        in_maps.append(
            {
                "x": xf[i * TOK : (i + 1) * TOK],
                "wt": wt,
                "wsl": wt[i * NS : (i + 1) * NS],
                "g": norm_weight,
            }
        )
    return in_maps


def kernel(x, weight, norm_weight):
    from concourse.bass_utils import run_bass_kernel_spmd

    in_maps = make_in_maps(x, weight, norm_weight)
    with_g = not bool(np.all(np.asarray(norm_weight) == 1.0))
    nc = _get_nc(with_g)
    res = run_bass_kernel_spmd(nc, in_maps, list(range(R)))
    y = np.concatenate([res.results[i]["y"] for i in range(R)], axis=0)
    return y.reshape(B, S, N)


if __name__ == "__main__":
    rng = np.random.default_rng(0)
    x = rng.standard_normal((B, S, D), dtype=np.float32)
    w = (rng.standard_normal((N, D), dtype=np.float32) * np.sqrt(2.0 / D)).astype(
        np.float32
    )
    g = np.ones(D, dtype=np.float32)
    y = kernel(x, w, g)
    print("ran", y.shape, y.dtype)


# revision 9
# speedup vs baseline: 1.2482x; 1.0331x over previous
"""BitLinear forward on 8 Trainium2 NeuronCores.

Reference computation (see harness reference.py):
    xn      = rmsnorm(x) * norm_weight                     # per token over D
    w_scale = 1 / max(mean(|W|), 1e-5)                     # global scalar
    w_q     = clip(round(W * w_scale), -1, 1)              # ternary
    x_scale = 127 / max(max|xn| per token, 1e-5)
    x_q     = clip(round(xn * x_scale), -128, 127)
    y       = (x_q @ w_q.T) / (w_scale * x_scale)

Distribution: data-parallel over tokens (1024/core), weight REPLICATED.
The host passes W pre-transposed (wt = W.T, [D, N] row-major) to every
core, so each core:
  - computes the global |W| abs-sum from its own disjoint 512-row slice
    of wt, AllReduces the scalar (the only collective on the critical
    path; a tiny warm-up AllReduce is issued at kernel start so the
    real one doesn't pay collective cold-start / launch skew),
  - rmsnorms + int8-quantizes + PE-transposes its 1024 tokens into a
    resident xqT [128, 32dc, 1024tok] bf16 SBUF tile, all before the
    matmul phase starts - no mid-matmul x stalls,
  - streams wt from its own DRAM in [128, 4dc, 512out] fp32 stages,
    quantizing to ternary bf16 on the fly (DVE round via the fp32
    +1.5*2^23 trick, then ACT Sign: clip(round(v),-1,1) == sign of the
    rounded integer; all ACT funcs used live in one table set),
  - runs 8 out-chunks x 8 token-tiles x 32 accumulating bf16 matmuls
    (exact integer arithmetic: x_q in [-127,127], w_q in {-1,0,1},
    fp32 PSUM partial sums < 2^24).
No AllGather, no quantized-W DRAM round-trip.
"""

import numpy as np

# ---------------------------------------------------------------- constants
R = 8  # cores
B, S, D = 4, 2048, 4096
N = 4096  # out features
TOK = (B * S) // R  # tokens per core (1024)
NS = N // R  # wslice rows per core (512)
HALF = D // 2  # x free-dim half tile (2048)
NTT = TOK // 128  # token tiles per core (8)
NDC = D // 128  # contraction chunks (32)
NCH = N // 512  # output chunks (8)
NST = 8  # W stages per chunk (4 dc each)
SDC = NDC // NST  # dc per stage (4)
C_ROUND = 12582912.0  # 1.5 * 2^23: fp32 add rounds to int, ties-to-even
EPS_NORM = 1e-5
Q_EPS = 1e-5

_CACHED = {}


def _legalize_waits(bir_bytes):
    """Split multi-wait BIR instructions into single-wait EventSemaphore
    chains: the walrus build here accepts at most one sync-wait command per
    instruction, while Tile's sem-assignment emits multi-wait joins."""
    import json

    bir = json.loads(bir_bytes)
    for fn in bir.get("functions", []):
        for bb in fn.get("blocks", []):
            new_insts = []
            for inst in bb.get("instructions", []):
                si = inst.get("sync_info")
                waits = (si or {}).get("on_wait") or []
                if len(waits) > 1:
                    movable = [w for w in waits if w.get("sync_type") == "semaphore"]
                    fixed = [w for w in waits if w.get("sync_type") != "semaphore"]
                    keep, hoist = (
                        (fixed, movable) if fixed else ([movable[-1]], movable[:-1])
                    )
                    if len(keep) > 1:
                        raise RuntimeError(
                            f"{inst.get('name')}: {len(keep)} non-hoistable waits"
                        )
                    for k, w in enumerate(hoist):
                        new_insts.append(
                            {
                                "debug": inst.get("debug", 0),
                                "engine": inst["engine"],
                                "ins": [],
                                "name": f"{inst['name']}_hw{k}",
                                "opcode": "EventSemaphore",
                                "outs": [],
                                "sync_info": {"on_update": [], "on_wait": [w]},
                            }
                        )
                    si["on_wait"] = keep
                new_insts.append(inst)
            bb["instructions"] = new_insts
    return json.dumps(bir).encode()


def _build(with_g):
    import concourse.bass as bass
    import concourse.mybir as mybir
    import concourse.tile as tile
    from concourse.bass import ts
    from concourse.masks import make_identity

    f32 = mybir.dt.float32
    bf16 = mybir.dt.bfloat16
    MULT = mybir.AluOpType.mult
    ADD = mybir.AluOpType.add
    MAX = mybir.AluOpType.max
    X_AX = mybir.AxisListType.X
    AF = mybir.ActivationFunctionType
    GROUP = [list(range(R))]

    nc = bass.Bass()
    xp = nc.declare_dram_parameter("x", [TOK, D], f32, isOutput=False)
    wtp = nc.declare_dram_parameter("wt", [D, N], f32, isOutput=False)
    wsl = nc.declare_dram_parameter("wsl", [NS, D], f32, isOutput=False)
    gp = nc.declare_dram_parameter("g", [D], f32, isOutput=False)
    yp = nc.declare_dram_parameter("y", [TOK, N], f32, isOutput=True)

    wcb_bufs = 12 if with_g else 16
    iox_bufs = 2 if with_g else 3

    with tile.TileContext(nc) as tc:
        with (
            tc.tile_pool(name="persist", bufs=1) as pp,
            tc.tile_pool(name="io_x", bufs=iox_bufs) as io_x,
            tc.tile_pool(name="scr_x", bufs=2) as scr_x,
            tc.tile_pool(name="wst", bufs=3) as wst,
            tc.tile_pool(name="wcb", bufs=wcb_bufs) as wcb_pool,
            tc.tile_pool(name="small", bufs=24) as sp,
            tc.tile_pool(name="yout", bufs=2) as ypool,
            tc.tile_pool(name="ps_t", bufs=2, space="PSUM") as ps_t,
            tc.tile_pool(name="ps_mm", bufs=5, space="PSUM") as ps_mm,
            tc.tile_pool(name="ps_s", bufs=1, space="PSUM") as ps_s,
            tc.tile_pool(name="dram", bufs=1, space="DRAM") as dram,
        ):
            # ---- persistent tiles
            xqT = pp.tile([128, NDC, TOK], bf16, name="xqT")
            ident = pp.tile([128, 128], f32, name="ident")
            make_identity(nc, ident[:])
            ones_col = pp.tile([128, 1], f32, name="ones_col")
            nc.vector.memset(ones_col[:], 1.0)
            amc_all = pp.tile([128, NTT], f32, name="amc_all")
            c2_all = pp.tile([128, NTT], f32, name="c2_all")
            s_rep = pp.tile([128, 1], f32, name="s_rep")
            m_rep = pp.tile([128, 1], f32, name="m_rep")
            wsc_rep = pp.tile([128, 1], f32, name="wsc_rep")
            dq_rep = pp.tile([128, 1], f32, name="dq_rep")
            eps_rep = pp.tile([128, 1], f32, name="eps_rep")
            nc.vector.memset(eps_rep[:], EPS_NORM)
            nround_rep = pp.tile([128, 1], f32, name="nround_rep")
            nc.vector.memset(nround_rep[:], -C_ROUND)
            if with_g:
                g_rep = pp.tile([128, D], f32, name="g_rep")
                nc.sync.dma_start(g_rep[:], gp[:].to_broadcast([128, D]))

            # ---- DRAM scratch
            ws_in = dram.tile([1, 1], f32, name="ws_in")
            ws_out = dram.tile([1, 1], f32, addr_space="Shared", name="ws_out")

            # ---- W1: partial |W| abs-sum over this core's disjoint slice
            parts = []
            for i in range(NS // 128):
                for h in range(2):
                    w_t = wst.tile([128, SDC, 512], f32, tag="wst", name="ws_t")
                    fl = w_t[:].rearrange("p j q -> p (j q)")
                    nc.sync.dma_start(fl, wsl[ts(i, 128), ts(h, HALF)])
                    part = sp.tile([128, 1], f32, tag="sm", name="part")
                    nc.scalar.activation(fl, fl, AF.Abs, accum_out=part[:])
                    parts.append(part)

            # ---- X(t): rmsnorm + int8 quantize + transpose into xqT
            def emit_x(t):
                srcs = []
                mss, amaxs = [], []
                for h in range(2):
                    x_t = io_x.tile([128, HALF], f32, tag="iox", name="x_t")
                    nc.sync.dma_start(x_t[:], xp[ts(t, 128), ts(h, HALF)])
                    ms_h = sp.tile([128, 1], f32, tag="sm", name="ms_h")
                    sq = scr_x.tile([128, HALF], f32, tag="scx", name="sq")
                    # sq <- x*x (scratch, overwritten later), ms_h <- sum(x^2)
                    nc.scalar.activation(sq[:], x_t[:], AF.Square, accum_out=ms_h[:])
                    if with_g:
                        nc.vector.tensor_mul(sq[:], x_t[:], g_rep[:, ts(h, HALF)])
                        src = sq
                    else:
                        src = x_t
                    srcs.append((src, sq))
                    am_h = sp.tile([128, 1], f32, tag="sm", name="am_h")
                    nc.vector.tensor_reduce(
                        am_h[:], src[:], axis=X_AX, op=MAX, apply_absolute_value=True
                    )
                    mss.append(ms_h)
                    amaxs.append(am_h)
                ms = sp.tile([128, 1], f32, tag="sm", name="ms")
                nc.vector.tensor_add(ms[:], mss[0][:], mss[1][:])
                amax = sp.tile([128, 1], f32, tag="sm", name="amax")
                nc.vector.tensor_tensor(amax[:], amaxs[0][:], amaxs[1][:], op=MAX)
                # r = 1/sqrt(ms/D + eps)
                sdev = sp.tile([128, 1], f32, tag="sm", name="sdev")
                nc.scalar.activation(
                    sdev[:], ms[:], AF.Sqrt, bias=eps_rep[:], scale=1.0 / D
                )
                r = sp.tile([128, 1], f32, tag="sm", name="r")
                nc.vector.reciprocal(r[:], sdev[:])
                # amc = max(amax*r, eps) = max(max|xn|, eps);  c = r*127/amc
                amn = sp.tile([128, 1], f32, tag="sm", name="amn")
                nc.vector.tensor_mul(amn[:], amax[:], r[:])
                nc.vector.tensor_scalar_max(amc_all[:, t : t + 1], amn[:], Q_EPS)
                inv = sp.tile([128, 1], f32, tag="sm", name="inv")
                nc.vector.reciprocal(inv[:], amc_all[:, t : t + 1])
                rc = sp.tile([128, 1], f32, tag="sm", name="rc")
                nc.vector.tensor_mul(rc[:], r[:], inv[:])
                c_t = sp.tile([128, 1], f32, tag="sm", name="c_t")
                nc.vector.tensor_scalar_mul(c_t[:], rc[:], 127.0)
                for h in range(2):
                    src, sq = srcs[h]
                    # v = src*c + C_ROUND  (fp32: rounds to int, ties-even)
                    nc.vector.tensor_scalar(
                        sq[:], src[:], c_t[:], C_ROUND, op0=MULT, op1=ADD
                    )
                    # transpose v (fp32) via PE; subtract C_ROUND during the
                    # PSUM->SBUF copy (ACT, fp32->bf16): xqT gets exact int8
                    for bk in range(4):
                        pst = ps_t.tile([128, 512], f32, tag="pst", name="pstx")
                        for j4 in range(4):
                            j = bk * 4 + j4
                            nc.tensor.transpose(
                                pst[:, ts(j4, 128)], sq[:, ts(j, 128)], ident[:]
                            )
                        dc0 = h * (HALF // 128) + bk * 4
                        nc.scalar.activation(
                            xqT[:, dc0 : dc0 + 4, ts(t, 128)],
                            pst[:].rearrange("p (j q) -> p j q", j=4),
                            AF.Copy,
                            bias=-C_ROUND,
                        )

            # ---- Wq(c): stream chunk c of wt, quantize to ternary bf16
            def emit_wq(c):
                tiles = []
                for g in range(NST):
                    ws = wst.tile([128, SDC, 512], f32, tag="wst", name="ws_t")
                    src = wtp[ts(g, SDC * 128), ts(c, 512)].rearrange(
                        "(j p) q -> p j q", p=128
                    )
                    nc.sync.dma_start(ws[:], src)
                    flat = ws[:].rearrange("p j q -> p (j q)")
                    # v = w*wsc + C_ROUND (fp32 round); wq = Sign(v - C_ROUND)
                    nc.vector.tensor_scalar(
                        flat, flat, wsc_rep[:], C_ROUND, op0=MULT, op1=ADD
                    )
                    wq = wcb_pool.tile([128, SDC, 512], bf16, tag="wcb", name="wq")
                    nc.scalar.activation(
                        wq[:].rearrange("p j q -> p (j q)"),
                        flat,
                        AF.Sign,
                        bias=nround_rep[:],
                    )
                    tiles.append(wq)
                return tiles

            # ---- MM(c): 8 token tiles x 32 accumulating matmuls
            def emit_mm(c, wq_tiles):
                for t in range(NTT):
                    pmm = ps_mm.tile([128, 512], f32, tag="pmm", name="pmm")
                    for g in range(NST):
                        for j in range(SDC):
                            dc = g * SDC + j
                            nc.tensor.matmul(
                                pmm[:],
                                lhsT=xqT[:, dc, ts(t, 128)],
                                rhs=wq_tiles[g][:, j, :],
                                start=(dc == 0),
                                stop=(dc == NDC - 1),
                            )
                    y_sb = ypool.tile([128, 512], f32, tag="y", name="y_sb")
                    nc.scalar.activation(
                        y_sb[:], pmm[:], AF.Copy, scale=c2_all[:, t : t + 1]
                    )
                    nc.sync.dma_start(yp[ts(t, 128), ts(c, 512)], y_sb[:])

            # ================= emission schedule =================
            for t in range(3):
                emit_x(t)

            # finish the abs-sum -> scalar AllReduce (PE reduction emitted
            # here so the x0-2 transposes aren't stuck behind its sem wait)
            while len(parts) > 1:
                nxt = []
                for a, b_ in zip(parts[::2], parts[1::2]):
                    s2 = sp.tile([128, 1], f32, tag="sm", name="s2")
                    nc.vector.tensor_add(s2[:], a[:], b_[:])
                    nxt.append(s2)
                if len(parts) % 2:
                    nxt.append(parts[-1])
                parts = nxt
            pst_s = ps_s.tile([1, 1], f32, name="pst_s")
            nc.tensor.matmul(
                pst_s[:], lhsT=parts[0][:], rhs=ones_col[:], start=True, stop=True
            )
            sb_tot = sp.tile([1, 1], f32, tag="one", name="sb_tot")
            nc.scalar.copy(sb_tot[:], pst_s[:])
            nc.sync.dma_start(ws_in[:], sb_tot[:])
            nc.gpsimd.collective_compute(
                "AllReduce",
                ADD,
                replica_groups=GROUP,
                ins=[ws_in[:]],
                outs=[ws_out[:]],
            )
            nc.sync.dma_start(s_rep[:], ws_out[:].to_broadcast([128, 1]))

            for t in range(3, NTT):
                emit_x(t)

            # w_scale machinery (replicated per partition):
            #   m_rep  = max(mean|W|, Q_EPS)   (= 1/w_scale)
            #   wsc_rep= 1/m_rep               (= w_scale)
            #   dq_rep = m_rep/127             (= 1/(127*w_scale))
            nc.vector.tensor_scalar(
                m_rep[:], s_rep[:], 1.0 / (N * D), Q_EPS, op0=MULT, op1=MAX
            )
            nc.vector.reciprocal(wsc_rep[:], m_rep[:])
            nc.vector.tensor_scalar_mul(dq_rep[:], m_rep[:], 1.0 / 127.0)
            # c2 = 1/(w_scale*x_scale) per token tile
            for t in range(NTT):
                nc.vector.tensor_mul(
                    c2_all[:, t : t + 1], amc_all[:, t : t + 1], dq_rep[:]
                )

            wq_c = emit_wq(0)
            wq_n = emit_wq(1)
            for c in range(NCH):
                cur = wq_c
                wq_c = wq_n
                if c + 2 < NCH:
                    wq_n = emit_wq(c + 2)
                emit_mm(c, cur)

    orig = nc.to_json_bytes

    def patched():
        return _legalize_waits(orig())

    nc.to_json_bytes = patched
    return nc


def _get_nc(with_g):
    key = ("nc", with_g)
    if key not in _CACHED:
        _CACHED[key] = _build(with_g)
    return _CACHED[key]


def make_in_maps(x, weight, norm_weight):
    x = np.ascontiguousarray(x, dtype=np.float32)
    weight = np.ascontiguousarray(weight, dtype=np.float32)
    norm_weight = np.ascontiguousarray(norm_weight, dtype=np.float32)
    xf = x.reshape(B * S, D)
    wt = np.ascontiguousarray(weight.T)
    in_maps = []
    for i in range(R):
        in_maps.append(
            {
                "x": xf[i * TOK : (i + 1) * TOK],
                "wt": wt,
                "wsl": wt[i * NS : (i + 1) * NS],
                "g": norm_weight,
            }
        )
    return in_maps


def kernel(x, weight, norm_weight):
    from concourse.bass_utils import run_bass_kernel_spmd

    in_maps = make_in_maps(x, weight, norm_weight)
    with_g = not bool(np.all(np.asarray(norm_weight) == 1.0))
    nc = _get_nc(with_g)
    res = run_bass_kernel_spmd(nc, in_maps, list(range(R)))
    y = np.concatenate([res.results[i]["y"] for i in range(R)], axis=0)
    return y.reshape(B, S, N)


if __name__ == "__main__":
    rng = np.random.default_rng(0)
    x = rng.standard_normal((B, S, D), dtype=np.float32)
    w = (rng.standard_normal((N, D), dtype=np.float32) * np.sqrt(2.0 / D)).astype(
        np.float32
    )
    g = np.ones(D, dtype=np.float32)
    y = kernel(x, w, g)
    print("ran", y.shape, y.dtype)


# revision 10
# speedup vs baseline: 1.2724x; 1.0193x over previous
"""BitLinear forward on 8 Trainium2 NeuronCores.

Reference computation (see harness reference.py):
    xn      = rmsnorm(x) * norm_weight                     # per token over D
    w_scale = 1 / max(mean(|W|), 1e-5)                     # global scalar
    w_q     = clip(round(W * w_scale), -1, 1)              # ternary
    x_scale = 127 / max(max|xn| per token, 1e-5)
    x_q     = clip(round(xn * x_scale), -128, 127)
    y       = (x_q @ w_q.T) / (w_scale * x_scale)

Distribution: data-parallel over tokens (1024/core), weight REPLICATED.
The host passes W pre-transposed (wt = W.T, [D, N] row-major) to every
core, so each core:
  - computes the global |W| abs-sum from its own disjoint 512-row slice
    of wt, AllReduces the scalar (the only collective on the critical
    path; a tiny warm-up AllReduce is issued at kernel start so the
    real one doesn't pay collective cold-start / launch skew),
  - rmsnorms + int8-quantizes + PE-transposes its 1024 tokens into a
    resident xqT [128, 32dc, 1024tok] bf16 SBUF tile, all before the
    matmul phase starts - no mid-matmul x stalls,
  - streams wt from its own DRAM in [128, 4dc, 512out] fp32 stages,
    quantizing to ternary bf16 on the fly (DVE round via the fp32
    +1.5*2^23 trick, then ACT Sign: clip(round(v),-1,1) == sign of the
    rounded integer; all ACT funcs used live in one table set),
  - runs 8 out-chunks x 8 token-tiles x 32 accumulating bf16 matmuls
    (exact integer arithmetic: x_q in [-127,127], w_q in {-1,0,1},
    fp32 PSUM partial sums < 2^24).
No AllGather, no quantized-W DRAM round-trip.
"""

import numpy as np

# ---------------------------------------------------------------- constants
R = 8  # cores
B, S, D = 4, 2048, 4096
N = 4096  # out features
TOK = (B * S) // R  # tokens per core (1024)
NS = N // R  # wslice rows per core (512)
HALF = D // 2  # x free-dim half tile (2048)
NTT = TOK // 128  # token tiles per core (8)
NDC = D // 128  # contraction chunks (32)
NCH = N // 512  # output chunks (8)
NST = 8  # W stages per chunk (4 dc each)
SDC = NDC // NST  # dc per stage (4)
C_ROUND = 12582912.0  # 1.5 * 2^23: fp32 add rounds to int, ties-to-even
EPS_NORM = 1e-5
Q_EPS = 1e-5

_CACHED = {}


def _legalize_waits(bir_bytes):
    """Split multi-wait BIR instructions into single-wait EventSemaphore
    chains: the walrus build here accepts at most one sync-wait command per
    instruction, while Tile's sem-assignment emits multi-wait joins."""
    import json

    bir = json.loads(bir_bytes)
    for fn in bir.get("functions", []):
        for bb in fn.get("blocks", []):
            new_insts = []
            for inst in bb.get("instructions", []):
                si = inst.get("sync_info")
                waits = (si or {}).get("on_wait") or []
                if len(waits) > 1:
                    movable = [w for w in waits if w.get("sync_type") == "semaphore"]
                    fixed = [w for w in waits if w.get("sync_type") != "semaphore"]
                    keep, hoist = (
                        (fixed, movable) if fixed else ([movable[-1]], movable[:-1])
                    )
                    if len(keep) > 1:
                        raise RuntimeError(
                            f"{inst.get('name')}: {len(keep)} non-hoistable waits"
                        )
                    for k, w in enumerate(hoist):
                        new_insts.append(
                            {
                                "debug": inst.get("debug", 0),
                                "engine": inst["engine"],
                                "ins": [],
                                "name": f"{inst['name']}_hw{k}",
                                "opcode": "EventSemaphore",
                                "outs": [],
                                "sync_info": {"on_update": [], "on_wait": [w]},
                            }
                        )
                    si["on_wait"] = keep
                new_insts.append(inst)
            bb["instructions"] = new_insts
    return json.dumps(bir).encode()


def _build(with_g):
    import concourse.bass as bass
    import concourse.mybir as mybir
    import concourse.tile as tile
    from concourse.bass import ts
    from concourse.masks import make_identity

    f32 = mybir.dt.float32
    bf16 = mybir.dt.bfloat16
    MULT = mybir.AluOpType.mult
    ADD = mybir.AluOpType.add
    MAX = mybir.AluOpType.max
    X_AX = mybir.AxisListType.X
    AF = mybir.ActivationFunctionType
    GROUP = [list(range(R))]

    nc = bass.Bass()
    xp = nc.declare_dram_parameter("x", [TOK, D], f32, isOutput=False)
    wtp = nc.declare_dram_parameter("wt", [D, N], f32, isOutput=False)
    wsl = nc.declare_dram_parameter("wsl", [NS, D], f32, isOutput=False)
    gp = nc.declare_dram_parameter("g", [D], f32, isOutput=False)
    yp = nc.declare_dram_parameter("y", [TOK, N], f32, isOutput=True)

    wcb_bufs = 12 if with_g else 16
    iox_bufs = 2 if with_g else 3

    with tile.TileContext(nc) as tc:
        with (
            tc.tile_pool(name="persist", bufs=1) as pp,
            tc.tile_pool(name="io_x", bufs=iox_bufs) as io_x,
            tc.tile_pool(name="scr_x", bufs=2) as scr_x,
            tc.tile_pool(name="wst", bufs=3) as wst,
            tc.tile_pool(name="wcb", bufs=wcb_bufs) as wcb_pool,
            tc.tile_pool(name="small", bufs=24) as sp,
            tc.tile_pool(name="yout", bufs=2) as ypool,
            tc.tile_pool(name="ps_t", bufs=2, space="PSUM") as ps_t,
            tc.tile_pool(name="ps_mm", bufs=5, space="PSUM") as ps_mm,
            tc.tile_pool(name="ps_s", bufs=1, space="PSUM") as ps_s,
            tc.tile_pool(name="dram", bufs=1, space="DRAM") as dram,
        ):
            # ---- persistent tiles
            xqT = pp.tile([128, NDC, TOK], bf16, name="xqT")
            ident = pp.tile([128, 128], f32, name="ident")
            make_identity(nc, ident[:])
            ones_col = pp.tile([128, 1], f32, name="ones_col")
            nc.vector.memset(ones_col[:], 1.0)
            amc_all = pp.tile([128, NTT], f32, name="amc_all")
            c2_all = pp.tile([128, NTT], f32, name="c2_all")
            s_rep = pp.tile([128, 1], f32, name="s_rep")
            m_rep = pp.tile([128, 1], f32, name="m_rep")
            wsc_rep = pp.tile([128, 1], f32, name="wsc_rep")
            dq_rep = pp.tile([128, 1], f32, name="dq_rep")
            eps_rep = pp.tile([128, 1], f32, name="eps_rep")
            nc.vector.memset(eps_rep[:], EPS_NORM)
            nround_rep = pp.tile([128, 1], f32, name="nround_rep")
            nc.vector.memset(nround_rep[:], -C_ROUND)
            if with_g:
                g_rep = pp.tile([128, D], f32, name="g_rep")
                nc.sync.dma_start(g_rep[:], gp[:].to_broadcast([128, D]))

            # ---- DRAM scratch
            ws_in = dram.tile([1, 1], f32, name="ws_in")
            ws_out = dram.tile([1, 1], f32, addr_space="Shared", name="ws_out")

            # ---- W1: partial |W| abs-sum over this core's disjoint slice
            parts = []
            for i in range(NS // 128):
                for h in range(2):
                    w_t = wst.tile([128, SDC, 512], f32, tag="wst", name="ws_t")
                    fl = w_t[:].rearrange("p j q -> p (j q)")
                    nc.sync.dma_start(fl, wsl[ts(i, 128), ts(h, HALF)])
                    part = sp.tile([128, 1], f32, tag="sm", name="part")
                    nc.scalar.activation(fl, fl, AF.Abs, accum_out=part[:])
                    parts.append(part)

            # ---- X(t): rmsnorm + int8 quantize + transpose into xqT
            def emit_x(t):
                srcs = []
                mss, amaxs = [], []
                for h in range(2):
                    x_t = io_x.tile([128, HALF], f32, tag="iox", name="x_t")
                    nc.sync.dma_start(x_t[:], xp[ts(t, 128), ts(h, HALF)])
                    ms_h = sp.tile([128, 1], f32, tag="sm", name="ms_h")
                    sq = scr_x.tile([128, HALF], f32, tag="scx", name="sq")
                    # sq <- x*x (scratch, overwritten later), ms_h <- sum(x^2)
                    nc.scalar.activation(sq[:], x_t[:], AF.Square, accum_out=ms_h[:])
                    if with_g:
                        nc.vector.tensor_mul(sq[:], x_t[:], g_rep[:, ts(h, HALF)])
                        src = sq
                    else:
                        src = x_t
                    srcs.append((src, sq))
                    am_h = sp.tile([128, 1], f32, tag="sm", name="am_h")
                    nc.vector.tensor_reduce(
                        am_h[:], src[:], axis=X_AX, op=MAX, apply_absolute_value=True
                    )
                    mss.append(ms_h)
                    amaxs.append(am_h)
                ms = sp.tile([128, 1], f32, tag="sm", name="ms")
                nc.vector.tensor_add(ms[:], mss[0][:], mss[1][:])
                amax = sp.tile([128, 1], f32, tag="sm", name="amax")
                nc.vector.tensor_tensor(amax[:], amaxs[0][:], amaxs[1][:], op=MAX)
                # r = 1/sqrt(ms/D + eps)
                sdev = sp.tile([128, 1], f32, tag="sm", name="sdev")
                nc.scalar.activation(
                    sdev[:], ms[:], AF.Sqrt, bias=eps_rep[:], scale=1.0 / D
                )
                r = sp.tile([128, 1], f32, tag="sm", name="r")
                nc.vector.reciprocal(r[:], sdev[:])
                # amc = max(amax*r, eps) = max(max|xn|, eps);  c = r*127/amc
                amn = sp.tile([128, 1], f32, tag="sm", name="amn")
                nc.vector.tensor_mul(amn[:], amax[:], r[:])
                nc.vector.tensor_scalar_max(amc_all[:, t : t + 1], amn[:], Q_EPS)
                inv = sp.tile([128, 1], f32, tag="sm", name="inv")
                nc.vector.reciprocal(inv[:], amc_all[:, t : t + 1])
                rc = sp.tile([128, 1], f32, tag="sm", name="rc")
                nc.vector.tensor_mul(rc[:], r[:], inv[:])
                c_t = sp.tile([128, 1], f32, tag="sm", name="c_t")
                nc.vector.tensor_scalar_mul(c_t[:], rc[:], 127.0)
                for h in range(2):
                    src, sq = srcs[h]
                    # v = src*c + C_ROUND  (fp32: rounds to int, ties-even)
                    nc.vector.tensor_scalar(
                        sq[:], src[:], c_t[:], C_ROUND, op0=MULT, op1=ADD
                    )
                    # transpose v (fp32) via PE; subtract C_ROUND during the
                    # PSUM->SBUF copy (ACT, fp32->bf16): xqT gets exact int8
                    for bk in range(4):
                        pst = ps_t.tile([128, 512], f32, tag="pst", name="pstx")
                        for j4 in range(4):
                            j = bk * 4 + j4
                            nc.tensor.transpose(
                                pst[:, ts(j4, 128)], sq[:, ts(j, 128)], ident[:]
                            )
                        dc0 = h * (HALF // 128) + bk * 4
                        nc.scalar.activation(
                            xqT[:, dc0 : dc0 + 4, ts(t, 128)],
                            pst[:].rearrange("p (j q) -> p j q", j=4),
                            AF.Copy,
                            bias=-C_ROUND,
                        )

            # ---- Wq(c): stream chunk c of wt, quantize to ternary bf16
            def emit_wq(c):
                tiles = []
                for g in range(NST):
                    ws = wst.tile([128, SDC, 512], f32, tag="wst", name="ws_t")
                    src = wtp[ts(g, SDC * 128), ts(c, 512)].rearrange(
                        "(j p) q -> p j q", p=128
                    )
                    nc.sync.dma_start(ws[:], src)
                    flat = ws[:].rearrange("p j q -> p (j q)")
                    # v = w*wsc + C_ROUND (fp32 round); wq = Sign(v - C_ROUND)
                    nc.vector.tensor_scalar(
                        flat, flat, wsc_rep[:], C_ROUND, op0=MULT, op1=ADD
                    )
                    wq = wcb_pool.tile([128, SDC, 512], bf16, tag="wcb", name="wq")
                    nc.scalar.activation(
                        wq[:].rearrange("p j q -> p (j q)"),
                        flat,
                        AF.Sign,
                        bias=nround_rep[:],
                    )
                    tiles.append(wq)
                return tiles

            # ---- MM(c): 8 token tiles x 32 accumulating matmuls
            def emit_mm(c, wq_tiles):
                for t in range(NTT):
                    pmm = ps_mm.tile([128, 512], f32, tag="pmm", name="pmm")
                    for g in range(NST):
                        for j in range(SDC):
                            dc = g * SDC + j
                            nc.tensor.matmul(
                                pmm[:],
                                lhsT=xqT[:, dc, ts(t, 128)],
                                rhs=wq_tiles[g][:, j, :],
                                start=(dc == 0),
                                stop=(dc == NDC - 1),
                            )
                    y_sb = ypool.tile([128, 512], f32, tag="y", name="y_sb")
                    nc.scalar.activation(
                        y_sb[:], pmm[:], AF.Copy, scale=c2_all[:, t : t + 1]
                    )
                    nc.sync.dma_start(yp[ts(t, 128), ts(c, 512)], y_sb[:])

            # ================= emission schedule =================
            for t in range(3):
                emit_x(t)

            # finish the abs-sum -> scalar AllReduce (PE reduction emitted
            # here so the x0-2 transposes aren't stuck behind its sem wait)
            while len(parts) > 1:
                nxt = []
                for a, b_ in zip(parts[::2], parts[1::2]):
                    s2 = sp.tile([128, 1], f32, tag="sm", name="s2")
                    nc.vector.tensor_add(s2[:], a[:], b_[:])
                    nxt.append(s2)
                if len(parts) % 2:
                    nxt.append(parts[-1])
                parts = nxt
            pst_s = ps_s.tile([1, 1], f32, name="pst_s")
            nc.tensor.matmul(
                pst_s[:], lhsT=parts[0][:], rhs=ones_col[:], start=True, stop=True
            )
            sb_tot = sp.tile([1, 1], f32, tag="one", name="sb_tot")
            nc.scalar.copy(sb_tot[:], pst_s[:])
            nc.sync.dma_start(ws_in[:], sb_tot[:])
            nc.gpsimd.collective_compute(
                "AllReduce",
                ADD,
                replica_groups=GROUP,
                ins=[ws_in[:]],
                outs=[ws_out[:]],
            )
            nc.sync.dma_start(s_rep[:], ws_out[:].to_broadcast([128, 1]))

            for t in range(3, NTT):
                emit_x(t)

            # w_scale machinery (replicated per partition):
            #   m_rep  = max(mean|W|, Q_EPS)   (= 1/w_scale)
            #   wsc_rep= 1/m_rep               (= w_scale)
            #   dq_rep = m_rep/127             (= 1/(127*w_scale))
            # tile_wait_until: keep every AR-gated op AFTER all x-prep work
            # in the static per-engine order, so the AR sem-wait can't
            # head-of-line-block the x5-7 quantize on DVE/ACT.
            with tc.tile_wait_until(0.16):
                nc.vector.tensor_scalar(
                    m_rep[:], s_rep[:], 1.0 / (N * D), Q_EPS, op0=MULT, op1=MAX
                )
                nc.vector.reciprocal(wsc_rep[:], m_rep[:])
                nc.vector.tensor_scalar_mul(dq_rep[:], m_rep[:], 1.0 / 127.0)
                # c2 = 1/(w_scale*x_scale) per token tile
                for t in range(NTT):
                    nc.vector.tensor_mul(
                        c2_all[:, t : t + 1], amc_all[:, t : t + 1], dq_rep[:]
                    )

                wq_c = emit_wq(0)
                wq_n = emit_wq(1)
            for c in range(NCH):
                cur = wq_c
                wq_c = wq_n
                if c + 2 < NCH:
                    wq_n = emit_wq(c + 2)
                emit_mm(c, cur)

    orig = nc.to_json_bytes

    def patched():
        return _legalize_waits(orig())

    nc.to_json_bytes = patched
    return nc


def _get_nc(with_g):
    key = ("nc", with_g)
    if key not in _CACHED:
        _CACHED[key] = _build(with_g)
    return _CACHED[key]


def make_in_maps(x, weight, norm_weight):
    x = np.ascontiguousarray(x, dtype=np.float32)
    weight = np.ascontiguousarray(weight, dtype=np.float32)
    norm_weight = np.ascontiguousarray(norm_weight, dtype=np.float32)
    xf = x.reshape(B * S, D)
    wt = np.ascontiguousarray(weight.T)
    in_maps = []
    for i in range(R):
        in_maps.append(
            {
                "x": xf[i * TOK : (i + 1) * TOK],
                "wt": wt,
                "wsl": wt[i * NS : (i + 1) * NS],
                "g": norm_weight,
            }
        )
    return in_maps


def kernel(x, weight, norm_weight):
    from concourse.bass_utils import run_bass_kernel_spmd

    in_maps = make_in_maps(x, weight, norm_weight)
    with_g = not bool(np.all(np.asarray(norm_weight) == 1.0))
    nc = _get_nc(with_g)
    res = run_bass_kernel_spmd(nc, in_maps, list(range(R)))
    y = np.concatenate([res.results[i]["y"] for i in range(R)], axis=0)
    return y.reshape(B, S, N)


if __name__ == "__main__":
    rng = np.random.default_rng(0)
    x = rng.standard_normal((B, S, D), dtype=np.float32)
    w = (rng.standard_normal((N, D), dtype=np.float32) * np.sqrt(2.0 / D)).astype(
        np.float32
    )
    g = np.ones(D, dtype=np.float32)
    y = kernel(x, w, g)
    print("ran", y.shape, y.dtype)
